# revision 13
# baseline (speedup 1.0000x reference)
"""Blockwise (compressed-KV) attention on 8 Trainium2 NeuronCores.

Problem: q,k,v [B=4,H=16,T=4096,D=128] fp32, BS=32.
  k_cmp/v_cmp = blockwise mean-pool of k/v along T -> [B,H,C=128,D]
  score = softmax(q @ k_cmp^T / sqrt(D))   [B,H,T,C]
  out   = score @ v_cmp                    [B,H,T,D]
Returns (out, score), matching the reference.

Sharding: the 64 (b,h) pairs are split 8-per-core (pure data parallel, no
communication).  Each core runs an identical Bass/Tile program over its
8 heads.

v3 dataflow per head on one core:
  DMA: whole-head 2 MiB transfers.  k/v loads ride the SP HWDGE ring,
  q loads + score stores ride the GpSimd SWDGE ring, out stores ride the
  ACT HWDGE ring -- three DMA paths drain in parallel.
  compression (fp32): 32 chunk matmuls per tensor with a [128,4] pooling
  matrix -> k_cmp^T | v_cmp^T in one PSUM bank.
  main loop over 8 subtiles of 512 q rows:
    DVE-cast q to bf16, 4 PE transposes -> q^T (PSUM), DVE evac to SBUF,
    S^T = one matmul (k_cmp^T stationary, q^T moving, N=512),
    exp via ScalarE once (scale=1/sqrt(D) folded) -> exp(S^T) bf16 SBUF,
    row sums on PE (exp(S^T) chunks x ones vector) -> PSUM, DVE reciprocal,
    4 PE transposes of exp(S^T) -> exp [t,c] in PSUM,
    normalize score from PSUM with recip (split ACT/DVE), store,
    PV as 4 matmuls (exp(S^T) chunk stationary, v_cmp moving),
    evacuate out with recip folded in (split ACT/DVE), store.
"""
import math

import numpy as np

import concourse.bass as bass
import concourse.tile as tile
from concourse import mybir
from concourse.bass_utils import run_bass_kernel_spmd
from concourse.vector_clock import ScopedClock

B, H, T, D = 4, 16, 4096, 128
BS_EXPECTED = 32
C = T // BS_EXPECTED  # 128 compressed slots
N_CORES = 8
HEADS_PER_CORE = B * H // N_CORES  # 8
N_SUB = T // 512  # 8 subtiles of 512 rows per head
F32 = mybir.dt.float32

# ---------------------------------------------------------------------------
# walrus in this toolchain rejects instructions carrying more than one sync
# wait.  Tile's scheduler freely emits several waits per instruction, and the
# kernel-tail drain accumulates one wait per outstanding semaphore.  Hoist all
# but one wait of every instruction onto dedicated same-engine NOPs placed
# immediately before it (same-engine program order keeps the semantics).
_MAX_WAITS = 1
_split_counter = [0]


def _split_multi_waits(ordered):
    for insts in ordered.values():
        expanded = []
        for inst in insts:
            si = inst.sync_info
            if si is not None and len(si.on_wait) > _MAX_WAITS:
                waits = list(si.on_wait)
                head, keep = waits[:-_MAX_WAITS], waits[-_MAX_WAITS:]
                for w in head:
                    _split_counter[0] += 1
                    expanded.append(mybir.InstNoOp(
                        name=f"waitsplit_{_split_counter[0]}",
                        ins=[], outs=[],
                        engine=inst.engine,
                        sync_info=mybir.SyncInfo(on_wait=[w], on_update=[]),
                        bass_nofuse=True,
                    ))
                inst.sync_info = mybir.SyncInfo(
                    on_wait=keep, on_update=list(si.on_update)
                )
            expanded.append(inst)
        insts[:] = expanded


_orig_lower_ordered = tile.TileContext._lower_ordered_insts


def _lower_ordered_split(self, ordered):
    _split_multi_waits(ordered)
    return _orig_lower_ordered(self, ordered)


tile.TileContext._lower_ordered_insts = _lower_ordered_split


def _drain_and_barrier_split(self, tick_clock, wait_clock):
    nc = self.nc
    drain_inst = nc.sync.drain()
    wait_clock.add_sem_waits(
        drain_inst.ins, ScopedClock({None: tick_clock.global_clock})
    )
    si = drain_inst.ins.sync_info
    waits = list(si.on_wait) if si is not None else []
    if len(waits) > _MAX_WAITS:
        drain_inst.ins.sync_info = mybir.SyncInfo(
            on_wait=waits[:_MAX_WAITS], on_update=list(si.on_update)
        )
        for i in range(_MAX_WAITS, len(waits), _MAX_WAITS):
            extra = nc.sync.drain()
            extra.ins.sync_info = mybir.SyncInfo(
                on_wait=waits[i : i + _MAX_WAITS], on_update=[]
            )
    nc.all_engine_barrier()
    assert self.sems is not None
    popped = nc._tile_sem_poison_stack.pop()
    assert popped is self._sem_poison
    nc.clear_and_free_semaphores(list(self.sems.allocated().values()))
    nc.all_engine_barrier()


tile.TileContext._drain_and_barrier = _drain_and_barrier_split
# ---------------------------------------------------------------------------


def build_program(reps: int = 1, dma_only: bool = False,
                  q_rings: str = "sp", score_ring: str = "pool",
                  out_ring: str = "act", store_bf16: bool = False) -> bass.Bass:
    """Build the per-core Bass program.  `reps` repeats the whole computation
    (identical work, same outputs) for slope-based wall-clock timing."""
    BF16 = mybir.dt.bfloat16
    nc = bass.Bass("TRN2", target_bir_lowering=False, debug=False,
                   num_devices=N_CORES)

    q_d = nc.dram_tensor("q", [HEADS_PER_CORE, T, D], F32, kind="ExternalInput").ap()
    k_d = nc.dram_tensor("k", [HEADS_PER_CORE, T, D], F32, kind="ExternalInput").ap()
    v_d = nc.dram_tensor("v", [HEADS_PER_CORE, T, D], F32, kind="ExternalInput").ap()
    ident_d = nc.dram_tensor("ident", [128, 128], F32, kind="ExternalInput").ap()
    pmat_d = nc.dram_tensor("pmat", [128, 4], F32, kind="ExternalInput").ap()
    out_d = nc.dram_tensor("out", [HEADS_PER_CORE, T, D], F32,
                           kind="ExternalOutput").ap()
    score_d = nc.dram_tensor("score", [HEADS_PER_CORE, T, C], F32,
                             kind="ExternalOutput").ap()

    inv_sqrt_d = 1.0 / math.sqrt(D)

    def ring(name):
        return {"pool": nc.gpsimd, "sp": nc.sync, "act": nc.scalar}[name]

    with tile.TileContext(nc) as tc:
        with (
            tc.tile_pool(name="singles", bufs=1) as singles,
            tc.tile_pool(name="kv", bufs=6) as kv_pool,
            tc.tile_pool(name="heads", bufs=2) as heads,
            tc.tile_pool(name="qsb", bufs=4) as qsb_pool,
            tc.tile_pool(name="sb", bufs=6) as sb_pool,
            tc.tile_pool(name="scoreP", bufs=3) as score_pool,
            tc.tile_pool(name="outP", bufs=3) as out_pool,
            tc.tile_pool(name="stream", bufs=2) as stream_pool,
            tc.tile_pool(name="small", bufs=4) as small_pool,
            tc.tile_pool(name="psA", bufs=2, space="PSUM") as psA,
            tc.tile_pool(name="psS", bufs=2, space="PSUM") as psS,
            tc.tile_pool(name="psST", bufs=2, space="PSUM") as psST,
            tc.tile_pool(name="psO", bufs=2, space="PSUM") as psO,
        ):
            ident = singles.tile([128, 128], F32)
            nc.sync.dma_start(out=ident, in_=ident_d)
            ident_bf = singles.tile([128, 128], BF16)
            nc.vector.tensor_copy(ident_bf, ident)
            pmat = singles.tile([128, 4], F32)
            nc.sync.dma_start(out=pmat, in_=pmat_d)
            ones_bf = singles.tile([128, 1], BF16)
            nc.vector.memset(ones_bf, 1.0)

            for _rep in range(reps):
                for h in range(HEADS_PER_CORE):
                    # ---- half-head 1 MiB loads, all on the SP ring
                    kv_halves = []
                    for half in range(2):
                        rows = slice(half * 2048, (half + 1) * 2048)
                        k_sb = kv_pool.tile([128, 16, D], F32, tag="kv")
                        nc.sync.dma_start(
                            out=k_sb,
                            in_=k_d[h, rows, :].rearrange("(j p) d -> p j d", p=128),
                        )
                        v_sb = kv_pool.tile([128, 16, D], F32, tag="kv")
                        nc.sync.dma_start(
                            out=v_sb,
                            in_=v_d[h, rows, :].rearrange("(j p) d -> p j d", p=128),
                        )
                        kv_halves.append((k_sb, v_sb))
                    q_halves = []
                    for half in range(2):
                        rows = slice(half * 2048, (half + 1) * 2048)
                        q_sb = qsb_pool.tile([128, 16, D], F32, tag="q")
                        ring(q_rings).dma_start(
                            out=q_sb,
                            in_=q_d[h, rows, :].rearrange("(j p) d -> p j d", p=128),
                        )
                        q_halves.append(q_sb)

                    if dma_only:
                        score_head = stream_pool.tile([128, 32, C], F32, tag="score")
                        nc.vector.memset(score_head[:, 0:1, 0:1], 0.5)
                        out_head = stream_pool.tile([128, 32, D], F32, tag="out")
                        nc.vector.memset(out_head[:, 0:1, 0:1], 0.25)
                        ring(score_ring).dma_start(
                            out=score_d[h].rearrange("(j p) c -> p j c", p=128),
                            in_=score_head,
                        )
                        ring(out_ring).dma_start(
                            out=out_d[h].rearrange("(j p) d -> p j d", p=128),
                            in_=out_head,
                        )
                        continue

                    # ---- compression (fp32): k_cmp^T | v_cmp^T in one bank
                    kcvc = psST.tile([128, 512], F32, tag="st")
                    for half in range(2):
                        k_sb, v_sb = kv_halves[half]
                        for j in range(16):
                            cc = 16 * half + j
                            nc.tensor.matmul(
                                kcvc[:, 4 * cc : 4 * cc + 4],
                                lhsT=k_sb[:, j, :], rhs=pmat,
                                start=True, stop=True,
                            )
                            nc.tensor.matmul(
                                kcvc[:, 256 + 4 * cc : 256 + 4 * cc + 4],
                                lhsT=v_sb[:, j, :], rhs=pmat,
                                start=True, stop=True,
                            )
                    k_cmpT = heads.tile([128, C], BF16, tag="kc")  # [d, c] bf16
                    nc.scalar.copy(k_cmpT, kcvc[:, 0:128])
                    v_cmpT = heads.tile([128, C], F32, tag="vt")  # [d, c] f32
                    nc.scalar.copy(v_cmpT, kcvc[:, 256:384])
                    vps = psO.tile([128, 128], F32, tag="o")
                    nc.tensor.transpose(vps, v_cmpT, ident)
                    v_cmp = heads.tile([128, D], BF16, tag="vc")  # [c, d] bf16
                    nc.scalar.copy(v_cmp, vps)

                    st_dt = BF16 if store_bf16 else F32

                    # ---- main loop: 2 halves x 4 subtiles of 512 q rows
                    for half in range(2):
                      score_hh = score_pool.tile([128, 16, C], st_dt, tag="score")
                      out_hh = out_pool.tile([128, 16, D], st_dt, tag="out")
                      score_flat = score_hh.rearrange("p j c -> p (j c)")
                      out_flat = out_hh.rearrange("p j d -> p (j d)")
                      for sub in range(4):
                        cols = slice(sub * 512, (sub + 1) * 512)
                        q_flat = q_halves[half].rearrange("p j d -> p (j d)")
                        qcols = cols
                        # q -> bf16, PE-transpose to q^T
                        q_bf = sb_pool.tile([128, 512], BF16, tag="qbf")
                        nc.vector.tensor_copy(q_bf, q_flat[:, qcols])
                        qT_ps = psA.tile([128, 512], BF16, tag="a")
                        for j in range(4):
                            nc.tensor.transpose(
                                qT_ps[:, 128 * j : 128 * (j + 1)],
                                q_bf[:, 128 * j : 128 * (j + 1)],
                                ident_bf,
                            )
                        qT = sb_pool.tile([128, 512], BF16, tag="qT")
                        nc.vector.tensor_copy(qT, qT_ps)

                        # S^T [c, t] in one matmul: k_cmp^T stationary,
                        # q^T streaming N=512
                        stp_ps = psST.tile([128, 512], F32, tag="st")
                        nc.tensor.matmul(
                            stp_ps, lhsT=k_cmpT, rhs=qT,
                            start=True, stop=True,
                        )
                        # exp(S^T) once -> bf16 SBUF (PV weights + score src)
                        expT_bf = sb_pool.tile([128, 512], BF16, tag="exp")
                        nc.scalar.activation(
                            expT_bf, stp_ps, mybir.ActivationFunctionType.Exp,
                            scale=inv_sqrt_d,
                        )
                        # row sums on PE: exp(S^T) chunk x ones -> [t, 1]
                        sums_ps = psST.tile([128, 4], F32, tag="st")
                        for j in range(4):
                            nc.tensor.matmul(
                                sums_ps[:, j : j + 1],
                                lhsT=expT_bf[:, 128 * j : 128 * (j + 1)],
                                rhs=ones_bf,
                                start=True, stop=True,
                            )
                        recip = small_pool.tile([128, 4], F32, tag="recip")
                        nc.vector.reciprocal(recip, sums_ps)

                        # exp back to [t, c]: 4 PE transposes
                        e_ps = psS.tile([128, 512], BF16, tag="s")
                        for j in range(4):
                            nc.tensor.transpose(
                                e_ps[:, 128 * j : 128 * (j + 1)],
                                expT_bf[:, 128 * j : 128 * (j + 1)],
                                ident_bf,
                            )
                        # normalize score from PSUM (split ACT/DVE)
                        score_half = score_flat[:, cols]
                        for j in range(4):
                            if j < 2:
                                nc.scalar.activation(
                                    score_half[:, 128 * j : 128 * (j + 1)],
                                    e_ps[:, 128 * j : 128 * (j + 1)],
                                    mybir.ActivationFunctionType.Copy,
                                    scale=recip[:, j : j + 1],
                                )
                            else:
                                nc.vector.tensor_scalar_mul(
                                    score_half[:, 128 * j : 128 * (j + 1)],
                                    e_ps[:, 128 * j : 128 * (j + 1)],
                                    recip[:, j : j + 1],
                                )

                        # PV on unnormalized exp^T; fold 1/rowsum into the
                        # PSUM evacuation (split ACT/DVE)
                        o_ps = psO.tile([128, 512], F32, tag="o")
                        for j in range(4):
                            nc.tensor.matmul(
                                o_ps[:, 128 * j : 128 * (j + 1)],
                                lhsT=expT_bf[:, 128 * j : 128 * (j + 1)],
                                rhs=v_cmp,
                                start=True, stop=True,
                            )
                        out_half = out_flat[:, cols]
                        for j in range(4):
                            if j < 2:
                                nc.scalar.activation(
                                    out_half[:, 128 * j : 128 * (j + 1)],
                                    o_ps[:, 128 * j : 128 * (j + 1)],
                                    mybir.ActivationFunctionType.Copy,
                                    scale=recip[:, j : j + 1],
                                )
                            else:
                                nc.vector.tensor_scalar_mul(
                                    out_half[:, 128 * j : 128 * (j + 1)],
                                    o_ps[:, 128 * j : 128 * (j + 1)],
                                    recip[:, j : j + 1],
                                )

                      # ---- half-head 1 MiB stores (SWDGE casts bf16->f32)
                      rows = slice(half * 2048, (half + 1) * 2048)
                      ring("pool" if store_bf16 else score_ring).dma_start(
                          out=score_d[h, rows, :].rearrange(
                              "(j p) c -> p j c", p=128),
                          in_=score_hh,
                      )
                      ring("pool" if store_bf16 else out_ring).dma_start(
                          out=out_d[h, rows, :].rearrange(
                              "(j p) d -> p j d", p=128),
                          in_=out_hh,
                      )
    return nc


def _make_const_inputs():
    ident = np.eye(128, dtype=np.float32)
    pmat = np.zeros((128, 4), dtype=np.float32)
    for t in range(128):
        pmat[t, t // 32] = 1.0 / 32.0
    return ident, pmat


_PROGRAM_CACHE: dict[int, bass.Bass] = {}


def kernel(q: np.ndarray, k: np.ndarray, v: np.ndarray, BS) -> tuple:
    assert int(BS) == BS_EXPECTED, f"kernel hardcodes BS=32, got {BS}"
    q = np.ascontiguousarray(np.asarray(q, dtype=np.float32)).reshape(B * H, T, D)
    k = np.ascontiguousarray(np.asarray(k, dtype=np.float32)).reshape(B * H, T, D)
    v = np.ascontiguousarray(np.asarray(v, dtype=np.float32)).reshape(B * H, T, D)

    if 1 not in _PROGRAM_CACHE:
        _PROGRAM_CACHE[1] = build_program(reps=1)
    nc = _PROGRAM_CACHE[1]

    ident, pmat = _make_const_inputs()
    in_maps = []
    for i in range(N_CORES):
        sl = slice(i * HEADS_PER_CORE, (i + 1) * HEADS_PER_CORE)
        in_maps.append({
            "q": q[sl], "k": k[sl], "v": v[sl],
            "ident": ident, "pmat": pmat,
        })

    res = run_bass_kernel_spmd(nc, in_maps, core_ids=list(range(N_CORES)))

    out = np.empty((B * H, T, D), dtype=np.float32)
    score = np.empty((B * H, T, C), dtype=np.float32)
    for i in range(N_CORES):
        sl = slice(i * HEADS_PER_CORE, (i + 1) * HEADS_PER_CORE)
        out[sl] = res.results[i]["out"]
        score[sl] = res.results[i]["score"]
    return out.reshape(B, H, T, D), score.reshape(B, H, T, C)
